# revision 9
# baseline (speedup 1.0000x reference)
"""Trainium2 8-core Bass kernel for nn_AttentionHPROJ (B=2,T=2048,C=1024,h=16,hd=64).

Sharding: core = 4*b + g owns batch b and heads [4g, 4g+4).
Matmul operands are 16-bit (1 cyc/row on the PE vs 2 cyc/row for f32r):
fp16 for x/Wqk/Wv and Q/K (logit precision), bf16 where exp outputs flow
(w_t, V, O, Wp — exp can reach ~e^18, overflowing fp16). PSUM accumulation
stays fp32. Layout is transposed (dout on partitions, tokens moving):
  - Q^T,K^T = wqk^T @ x^T ; V natural = x @ Wv^T (lhsT = x^T tiles)
  - S^T = K_h Q_h^T per (head, key-tile) into a single-bank PSUM tile;
    exp on ACT (PSUM->SBUF bf16); causal via triangular mask multiply on
    diagonal subtiles; softmax denominator via ones-column appended to V
    (row 64 of the O^T PSUM accumulator).
  - All 4 heads accumulate O^T concurrently (4 PSUM banks), so the chunk
    ends with ONE packed [4,512] reciprocal (DVE cost scales with free-dim
    length only) + per-head gpsimd broadcast + normalize multiply.
  - Each core emits partial Y^T [1024, 2048] (its 4 heads); host sums the
    quad per batch and transposes.
Phase-1 (QKV) work for chunk i+1 and the c_proj of chunk i-1 are spread
through attention block i as PE filler at single-matmul granularity so the
in-order tensor queue never stalls on the normalize chain and the HAM
clock gate stays at 8/8 (2.4 GHz).
"""
import sys

sys.path.insert(0, "/opt/trn_rl_repo")

import numpy as np

B, T, C = 2, 2048, 1024
NH, HD = 16, 64
P = 128
NCORE = 8
QC = 512          # q-chunk width
NQC = T // QC     # 4
KSUB = C // P     # 8

_CACHE = {}


def _build():
    import functools

    import concourse.bass as bass
    import concourse.mybir as mybir
    import concourse.tile as tile
    from concourse import bacc

    F32 = mybir.dt.float32
    BF16 = mybir.dt.bfloat16
    FP16 = mybir.dt.float16
    EXP = mybir.ActivationFunctionType.Exp
    MUL = mybir.AluOpType.mult

    import time as _time
    _t0 = _time.time()
    print("[build] start", flush=True)
    nc = bacc.Bacc("TRN2", target_bir_lowering=False, debug=False, num_devices=NCORE)

    xt_ext = nc.dram_tensor("xt", [C, T], FP16, kind="ExternalInput").ap()
    wqk_ext = nc.dram_tensor("wqk", [C, 512], FP16, kind="ExternalInput").ap()
    wv_ext = nc.dram_tensor("wv", [C, 256], FP16, kind="ExternalInput").ap()
    wp_ext = nc.dram_tensor("wp", [P, 2, C], BF16, kind="ExternalInput").ap()
    mask_ext = nc.dram_tensor("mask", [P, P], BF16, kind="ExternalInput").ap()
    out_ext = nc.dram_tensor("out", [C, T], F32, kind="ExternalOutput").ap()

    with tile.TileContext(nc) as tc:
        with (
            tc.tile_pool(name="wpool", bufs=1) as wpool,
            tc.tile_pool(name="xpool", bufs=1) as xpool,
            tc.tile_pool(name="qkpool", bufs=1) as qkpool,
            tc.tile_pool(name="vpool", bufs=1) as vpool,
            tc.tile_pool(name="wtile", bufs=6) as wtpool,
            tc.tile_pool(name="opool", bufs=2) as opool,
            tc.tile_pool(name="ypool", bufs=2) as ypool,
            tc.tile_pool(name="small", bufs=8) as small,
            tc.tile_pool(name="psA", bufs=2, space="PSUM") as psA,  # 2 banks: S
            tc.tile_pool(name="psO", bufs=4, space="PSUM") as psO,  # 4 banks: O
            tc.tile_pool(name="psB", bufs=2, space="PSUM") as psB,  # 2 banks: qkv/proj
        ):
            # ---- weights / constants / inputs ----
            wqk_sb = wpool.tile([P, KSUB, 512], FP16)
            wv_sb = wpool.tile([P, KSUB, 256], FP16)
            xt_t = [xpool.tile([P, KSUB, QC], FP16, name=f"xt_{i}", tag=f"xt_{i}")
                    for i in range(NQC)]
            # qk_t planes: 0,1 = Q heads (01),(23); 2,3 = K heads (01),(23);
            # within a plane, partitions 0-63 = even head, 64-127 = odd head.
            qk_t = [qkpool.tile([P, 4, QC], FP16, name=f"qk_{i}", tag=f"qk_{i}")
                    for i in range(NQC)]
            v_t = [vpool.tile([P, 4, 4 * 65], BF16, name=f"v_{i}", tag=f"v_{i}")
                   for i in range(NQC)]
            mask_sb = wpool.tile([P, P], BF16)
            wp_sb = wpool.tile([P, 2, C], BF16)
            den_t = wpool.tile([P, QC], F32)
            rec_t = wpool.tile([P, QC], F32)

            wqk_r = wqk_ext.rearrange("(ko p) m -> p ko m", p=P)
            wv_r = wv_ext.rearrange("(ko p) m -> p ko m", p=P)
            xt_r = xt_ext.rearrange("(ko p) t -> p ko t", p=P)
            out_r = out_ext.rearrange("(d p) t -> p d t", p=P)

            # Queue order = need order: tiny constants, then block-0 feed
            # (wqk interleaved with xt chunk 0), then wv/wp, then the rest.
            nc.sync.dma_start(mask_sb[:], mask_ext[:])
            nc.vector.memset(den_t[:], 1.0)
            for i in range(NQC):
                ov = v_t[i].rearrange("p t (h c) -> p t h c", c=65)[:, :, :, 64]
                nc.vector.memset(ov, 1.0)
            for k in range(KSUB):
                nc.sync.dma_start(wqk_sb[:, k], wqk_r[:, k])
                nc.sync.dma_start(xt_t[0][:, k], xt_r[:, k, 0:QC])
            nc.sync.dma_start(wv_sb[:], wv_r[:])
            nc.sync.dma_start(wp_sb[:], wp_ext[:])
            for i in range(1, NQC):
                nc.sync.dma_start(xt_t[i][:], xt_r[:, :, i * QC:(i + 1) * QC])

            # ---- phase-1 units (returned as closure lists for filling) ----
            def qk_unit(tc_i, dt):
                st = {}

                def mm(k):
                    if "ps" not in st:
                        st["ps"] = psB.tile([P, QC], F32,
                                            name=f"ps_qk_{tc_i}_{dt}", tag="B")
                    nc.tensor.matmul(
                        st["ps"][:],
                        lhsT=wqk_sb[:, k, dt * P:(dt + 1) * P],
                        rhs=xt_t[tc_i][:, k],
                        start=(k == 0),
                        stop=(k == KSUB - 1),
                    )

                def cp():
                    nc.vector.tensor_copy(out=qk_t[tc_i][:, dt], in_=st["ps"][:])

                return [functools.partial(mm, k) for k in range(KSUB)] + [cp]

            def v_unit(tc_i, tlp):
                st = {}

                def mm(d, k):
                    if "ps" not in st:
                        st["ps"] = psB.tile([P, 2, 256], F32,
                                            name=f"ps_v_{tc_i}_{tlp}", tag="B")
                    tl = 2 * tlp + d
                    nc.tensor.matmul(
                        st["ps"][:, d],
                        lhsT=xt_t[tc_i][:, k, tl * P:(tl + 1) * P],
                        rhs=wv_sb[:, k],
                        start=(k == 0),
                        stop=(k == KSUB - 1),
                    )

                def cp():
                    vdst = v_t[tc_i].rearrange("p t (h c) -> p t h c", c=65)[
                        :, 2 * tlp:2 * tlp + 2, :, 0:64
                    ]
                    nc.vector.tensor_copy(
                        out=vdst,
                        in_=st["ps"].rearrange("p t (h d) -> p t h d", d=64),
                    )

                return [functools.partial(mm, d, k)
                        for d in (0, 1) for k in range(KSUB)] + [cp]

            def phase1_fillers(tc_i):
                fs = []
                for dt in (0, 2, 1, 3):
                    fs += qk_unit(tc_i, dt)
                for tlp in (0, 1):
                    fs += v_unit(tc_i, tlp)
                return fs

            # ---- c_proj unit for one finished chunk (as filler closures) ----
            def proj_unit(qc, o_t):
                st = {}
                fs = []

                def mm(dt, pr):
                    if dt not in st:
                        st[dt] = psB.tile([P, QC], F32,
                                          name=f"ps_y_{qc}_{dt}", tag="B")
                    nc.tensor.matmul(
                        st[dt][:],
                        lhsT=wp_sb[:, pr, dt * P:(dt + 1) * P],
                        rhs=o_t[:, pr, :],
                        start=(pr == 0),
                        stop=(pr == 1),
                    )

                def cp(dt):
                    nc.vector.tensor_copy(out=st["y"][:, dt], in_=st[dt][:])

                def alloc_y():
                    st["y"] = ypool.tile([P, 8, QC], F32, name=f"y_{qc}", tag="Y")

                fs.append(alloc_y)
                for dt in range(8):
                    fs.append(functools.partial(mm, dt, 0))
                    fs.append(functools.partial(mm, dt, 1))
                    fs.append(functools.partial(cp, dt))

                def dma():
                    nc.sync.dma_start(out_r[:, :, qc * QC:(qc + 1) * QC], st["y"][:])

                fs.append(dma)
                return fs

            # ---- attention ----
            def attn_block(qc, fillers):
                o_t = opool.tile([P, 2, QC], BF16, name=f"o_{qc}", tag="OT")
                nkt = 4 * qc + 4
                nsteps = nkt + 1
                done = 0
                step = 0

                po = [psO.tile([P, QC], F32, name=f"ps_o_{qc}_{h}", tag="O")
                      for h in range(4)]
                for kt in range(nkt):
                    j = kt - 4 * qc
                    c0 = max(0, j) * P
                    kb, kl = kt // 4, kt % 4
                    for h in range(4):
                        pbase = 64 * (h % 2)
                        qplane = h // 2
                        kplane = 2 + h // 2
                        ps_s = psA.tile([P, QC], F32,
                                        name=f"ps_s_{qc}_{h}_{kt}", tag="A")
                        nc.tensor.matmul(
                            ps_s[:, c0:QC],
                            lhsT=qk_t[kb][pbase:pbase + 64, kplane,
                                          kl * P:(kl + 1) * P],
                            rhs=qk_t[qc][pbase:pbase + 64, qplane, c0:QC],
                            start=True,
                            stop=True,
                        )
                        w_t = wtpool.tile([P, QC], BF16, name="w_t", tag="W")
                        nc.scalar.activation(w_t[:, c0:], ps_s[:, c0:], EXP)
                        if j >= 0:
                            nc.vector.tensor_tensor(
                                out=w_t[:, c0:c0 + P],
                                in0=w_t[:, c0:c0 + P],
                                in1=mask_sb[:],
                                op=MUL,
                            )
                        nc.tensor.matmul(
                            po[h][0:65, c0:QC],
                            lhsT=v_t[kb][:, kl, 65 * h:65 * h + 65],
                            rhs=w_t[:, c0:QC],
                            start=(kt == 0),
                            stop=(kt == nkt - 1),
                        )
                    step += 1
                    want = step * len(fillers) // nsteps
                    while done < want:
                        fillers[done]()
                        done += 1

                # one packed reciprocal for all 4 heads of this chunk
                # (engine partition starts must be multiples of 32, so the 4
                # denominator rows sit at partitions 0/32/64/96 of a
                # persistent tile; unused partitions were memset once)
                for h in range(4):
                    nc.vector.tensor_copy(
                        out=den_t[32 * h:32 * h + 1], in_=po[h][64:65, :]
                    )
                nc.vector.reciprocal(rec_t[:], den_t[:])
                for h in range(4):
                    # gpsimd broadcast source must sit at partition 0 on HW
                    rr = small.tile([1, QC], F32, name="rr", tag="RR")
                    nc.vector.tensor_copy(out=rr[:], in_=rec_t[32 * h:32 * h + 1])
                    rbc = small.tile([64, QC], F32, name="rbc", tag="RB")
                    nc.gpsimd.partition_broadcast(rbc[:], rr[:])
                    nc.vector.tensor_tensor(
                        out=o_t[64 * (h % 2):64 * (h % 2) + 64, h // 2, :],
                        in0=po[h][0:64, :],
                        in1=rbc[:],
                        op=MUL,
                    )
                while done < len(fillers):
                    fillers[done]()
                    done += 1
                return o_t

            for f in phase1_fillers(0):
                f()
            prev = []  # proj fillers from the previous chunk
            for i in range(NQC):
                fillers = prev + (phase1_fillers(i + 1) if i + 1 < NQC else [])
                o_t = attn_block(i, fillers)
                prev = proj_unit(i, o_t)
            for f in prev:
                f()

    print(f"[build] traced+scheduled {_time.time()-_t0:.1f}s", flush=True)
    nc.compile()
    print(f"[build] compiled {_time.time()-_t0:.1f}s", flush=True)
    return nc


def _get_nc():
    if "nc" not in _CACHE:
        _CACHE["nc"] = _build()
    return _CACHE["nc"]


def _make_in_maps(x, W_attn, W_proj):
    import ml_dtypes

    BF = ml_dtypes.bfloat16
    Wp = W_proj.reshape(NH, C, HD)  # [head, dout, d]
    A = Wp.reshape(8, 2, C, HD)     # [pair, hl, dout, d]
    wp_all = np.ascontiguousarray(A.transpose(1, 3, 0, 2).reshape(P, 8, C))
    mask_host = np.triu(np.ones([P, P], dtype=np.float32)).astype(BF)

    in_maps = []
    for core in range(NCORE):
        b, g = core // 4, core % 4
        xt = np.ascontiguousarray(x[b].T).astype(np.float16)  # [C, T]
        Wq = W_attn[256 * g: 256 * (g + 1)]
        Wk = W_attn[C + 256 * g: C + 256 * (g + 1)]
        Wv = W_attn[2 * C + 256 * g: 2 * C + 256 * (g + 1)]
        wqk = np.ascontiguousarray(
            np.concatenate([Wq, Wk], 0).T).astype(np.float16)
        wv = np.ascontiguousarray(Wv.T).astype(np.float16)  # [C, 256]
        wp = np.ascontiguousarray(wp_all[:, 2 * g: 2 * g + 2, :]).astype(BF)
        in_maps.append(
            {"xt": xt, "wqk": wqk, "wv": wv, "wp": wp, "mask": mask_host}
        )
    return in_maps


def kernel(x, W_attn, W_proj):
    from concourse.bass_utils import run_bass_kernel_spmd

    x = np.asarray(x, dtype=np.float32)
    W_attn = np.asarray(W_attn, dtype=np.float32)
    W_proj = np.asarray(W_proj, dtype=np.float32)

    in_maps = _make_in_maps(x, W_attn, W_proj)
    nc = _get_nc()
    res = run_bass_kernel_spmd(nc, in_maps, core_ids=list(range(NCORE)))
    _CACHE["last_result"] = res

    Y = np.empty((B, T, C), dtype=np.float32)
    for b in range(B):
        acc = res.results[4 * b]["out"].astype(np.float32).copy()
        for g in range(1, 4):
            acc += res.results[4 * b + g]["out"]
        Y[b] = acc.T
    return Y


# revision 10
# speedup vs baseline: 1.2632x; 1.2632x over previous
"""Trainium2 8-core Bass kernel for nn_AttentionHPROJ (B=2,T=2048,C=1024,h=16,hd=64).

Sharding: core = 4*b + g owns batch b and heads [4g, 4g+4).
Matmul operands are 16-bit (1 cyc/row on the PE vs 2 cyc/row for f32r):
fp16 for x/Wqk/Wv and Q/K (logit precision), bf16 where exp outputs flow
(w_t, V, O, Wp — exp can reach ~e^18, overflowing fp16). PSUM accumulation
stays fp32. Layout is transposed (dout on partitions, tokens moving):
  - Q^T,K^T = wqk^T @ x^T ; V natural = x @ Wv^T (lhsT = x^T tiles)
  - S^T = K_h Q_h^T per (head, key-tile) into a single-bank PSUM tile;
    exp on ACT (PSUM->SBUF bf16); causal via triangular mask multiply on
    diagonal subtiles; softmax denominator via ones-column appended to V
    (row 64 of the O^T PSUM accumulator).
  - All 4 heads accumulate O^T concurrently (4 PSUM banks), so the chunk
    ends with ONE packed [4,512] reciprocal (DVE cost scales with free-dim
    length only) + per-head gpsimd broadcast + normalize multiply.
  - Each core emits partial Y^T [1024, 2048] (its 4 heads); host sums the
    quad per batch and transposes.
Phase-1 (QKV) work for chunk i+1 and the c_proj of chunk i-1 are spread
through attention block i as PE filler at single-matmul granularity so the
in-order tensor queue never stalls on the normalize chain and the HAM
clock gate stays at 8/8 (2.4 GHz).
"""
import sys

sys.path.insert(0, "/opt/trn_rl_repo")

import numpy as np

B, T, C = 2, 2048, 1024
NH, HD = 16, 64
P = 128
NCORE = 8
QC = 512          # q-chunk width
NQC = T // QC     # 4
KSUB = C // P     # 8

_CACHE = {}


def _build():
    import functools

    import concourse.bass as bass
    import concourse.mybir as mybir
    import concourse.tile as tile
    from concourse import bacc

    F32 = mybir.dt.float32
    BF16 = mybir.dt.bfloat16
    FP16 = mybir.dt.float16
    EXP = mybir.ActivationFunctionType.Exp
    LN = mybir.ActivationFunctionType.Ln
    MUL = mybir.AluOpType.mult

    import time as _time
    _t0 = _time.time()
    print("[build] start", flush=True)
    nc = bacc.Bacc("TRN2", target_bir_lowering=False, debug=False, num_devices=NCORE)

    xt_ext = nc.dram_tensor("xt", [C, T], FP16, kind="ExternalInput").ap()
    wqk_ext = nc.dram_tensor("wqk", [C, 512], FP16, kind="ExternalInput").ap()
    wv_ext = nc.dram_tensor("wv", [C, 256], FP16, kind="ExternalInput").ap()
    wp_ext = nc.dram_tensor("wp", [P, 2, C], BF16, kind="ExternalInput").ap()
    mask_ext = nc.dram_tensor("mask", [P, P], BF16, kind="ExternalInput").ap()
    out_ext = nc.dram_tensor("out", [C, T], F32, kind="ExternalOutput").ap()

    with tile.TileContext(nc) as tc:
        with (
            tc.tile_pool(name="wpool", bufs=1) as wpool,
            tc.tile_pool(name="xpool", bufs=1) as xpool,
            tc.tile_pool(name="qkpool", bufs=1) as qkpool,
            tc.tile_pool(name="vpool", bufs=1) as vpool,
            tc.tile_pool(name="wtile", bufs=6) as wtpool,
            tc.tile_pool(name="opool", bufs=2) as opool,
            tc.tile_pool(name="ypool", bufs=2) as ypool,
            tc.tile_pool(name="small", bufs=8) as small,
            tc.tile_pool(name="psA", bufs=2, space="PSUM") as psA,  # 4 banks: S pair
            tc.tile_pool(name="psO", bufs=1, space="PSUM") as psO,  # 2 banks: O pair
            tc.tile_pool(name="psB", bufs=2, space="PSUM") as psB,  # 2 banks: qkv/proj
        ):
            # ---- weights / constants / inputs ----
            wqk_sb = wpool.tile([P, KSUB, 512], FP16)
            wv_sb = wpool.tile([P, KSUB, 256], FP16)
            xt_t = [xpool.tile([P, KSUB, QC], FP16, name=f"xt_{i}", tag=f"xt_{i}")
                    for i in range(NQC)]
            # qk_t planes: 0,1 = Q heads (01),(23); 2,3 = K heads (01),(23);
            # within a plane, partitions 0-63 = even head, 64-127 = odd head.
            qk_t = [qkpool.tile([P, 4, QC], FP16, name=f"qk_{i}", tag=f"qk_{i}")
                    for i in range(NQC)]
            v_t = [vpool.tile([P, 4, 4 * 65], BF16, name=f"v_{i}", tag=f"v_{i}")
                   for i in range(NQC)]
            mask_sb = wpool.tile([P, P], BF16)
            wp_sb = wpool.tile([P, 2, C], BF16)

            wqk_r = wqk_ext.rearrange("(ko p) m -> p ko m", p=P)
            wv_r = wv_ext.rearrange("(ko p) m -> p ko m", p=P)
            xt_r = xt_ext.rearrange("(ko p) t -> p ko t", p=P)
            out_r = out_ext.rearrange("(d p) t -> p d t", p=P)

            # Queue order = need order: tiny constants, then block-0 feed
            # (wqk interleaved with xt chunk 0), then wv/wp, then the rest.
            nc.sync.dma_start(mask_sb[:], mask_ext[:])
            for i in range(NQC):
                ov = v_t[i].rearrange("p t (h c) -> p t h c", c=65)[:, :, :, 64]
                nc.vector.memset(ov, 1.0)
            for k in range(KSUB):
                nc.sync.dma_start(wqk_sb[:, k], wqk_r[:, k])
                nc.sync.dma_start(xt_t[0][:, k], xt_r[:, k, 0:QC])
            nc.sync.dma_start(wv_sb[:], wv_r[:])
            nc.sync.dma_start(wp_sb[:], wp_ext[:])
            for i in range(1, NQC):
                nc.sync.dma_start(xt_t[i][:], xt_r[:, :, i * QC:(i + 1) * QC])

            # ---- phase-1 units (returned as closure lists for filling) ----
            def qk_unit(tc_i, dt):
                st = {}

                def mm(k):
                    if "ps" not in st:
                        st["ps"] = psB.tile([P, QC], F32,
                                            name=f"ps_qk_{tc_i}_{dt}", tag="B")
                    nc.tensor.matmul(
                        st["ps"][:],
                        lhsT=wqk_sb[:, k, dt * P:(dt + 1) * P],
                        rhs=xt_t[tc_i][:, k],
                        start=(k == 0),
                        stop=(k == KSUB - 1),
                    )

                def cp():
                    nc.vector.tensor_copy(out=qk_t[tc_i][:, dt], in_=st["ps"][:])

                return [functools.partial(mm, k) for k in range(KSUB)] + [cp]

            def v_unit(tc_i, tlp):
                st = {}

                def mm(d, k):
                    if "ps" not in st:
                        st["ps"] = psB.tile([P, 2, 256], F32,
                                            name=f"ps_v_{tc_i}_{tlp}", tag="B")
                    tl = 2 * tlp + d
                    nc.tensor.matmul(
                        st["ps"][:, d],
                        lhsT=xt_t[tc_i][:, k, tl * P:(tl + 1) * P],
                        rhs=wv_sb[:, k],
                        start=(k == 0),
                        stop=(k == KSUB - 1),
                    )

                def cp():
                    vdst = v_t[tc_i].rearrange("p t (h c) -> p t h c", c=65)[
                        :, 2 * tlp:2 * tlp + 2, :, 0:64
                    ]
                    nc.vector.tensor_copy(
                        out=vdst,
                        in_=st["ps"].rearrange("p t (h d) -> p t h d", d=64),
                    )

                return [functools.partial(mm, d, k)
                        for d in (0, 1) for k in range(KSUB)] + [cp]

            def phase1_fillers(tc_i):
                fs = []
                for dt in (0, 2, 1, 3):
                    fs += qk_unit(tc_i, dt)
                for tlp in (0, 1):
                    fs += v_unit(tc_i, tlp)
                return fs

            # ---- c_proj unit for one finished chunk (as filler closures) ----
            def proj_unit(qc, o_t):
                st = {}
                fs = []

                def mm(dt, pr):
                    if dt not in st:
                        st[dt] = psB.tile([P, QC], F32,
                                          name=f"ps_y_{qc}_{dt}", tag="B")
                    nc.tensor.matmul(
                        st[dt][:],
                        lhsT=wp_sb[:, pr, dt * P:(dt + 1) * P],
                        rhs=o_t[:, pr, :],
                        start=(pr == 0),
                        stop=(pr == 1),
                    )

                def cp(dt):
                    nc.vector.tensor_copy(out=st["y"][:, dt], in_=st[dt][:])

                def alloc_y():
                    st["y"] = ypool.tile([P, 8, QC], F32, name=f"y_{qc}", tag="Y")

                fs.append(alloc_y)
                for dt in range(8):
                    fs.append(functools.partial(mm, dt, 0))
                    fs.append(functools.partial(mm, dt, 1))
                    fs.append(functools.partial(cp, dt))

                def dma():
                    nc.sync.dma_start(out_r[:, :, qc * QC:(qc + 1) * QC], st["y"][:])

                fs.append(dma)
                return fs

            # ---- attention ----
            # hp pair loop: S for both heads of a partition-pair land in one
            # [P,2,QC] PSUM mega-tile (2 banks) -> ONE 1024-wide exp; O^T for
            # the pair accumulates in a [P,2,QC] pair-mega; the softmax
            # denominators (row 64 of both planes) are inverted entirely on
            # ACT via exp(-ln(x)) read straight from PSUM.
            def attn_block(qc, fillers):
                o_t = opool.tile([P, 2, QC], BF16, name=f"o_{qc}", tag="OT")
                nkt = 4 * qc + 4
                nsteps = 2 * (nkt + 1)
                done = 0
                step = 0

                def fill():
                    nonlocal done
                    want = step * len(fillers) // nsteps
                    while done < want:
                        fillers[done]()
                        done += 1

                for hp in range(2):
                    po = psO.tile([P, 2, QC], F32, name=f"ps_o_{qc}_{hp}", tag="O")
                    for kt in range(nkt):
                        j = kt - 4 * qc
                        c0 = max(0, j) * P
                        kb, kl = kt // 4, kt % 4
                        ps_s = psA.tile([P, 2, QC], F32,
                                        name=f"ps_s_{qc}_{hp}_{kt}", tag="A")
                        for d in (0, 1):
                            h = 2 * hp + d
                            pbase = 64 * (h % 2)
                            qplane = h // 2
                            kplane = 2 + h // 2
                            nc.tensor.matmul(
                                ps_s[:, d, c0:QC],
                                lhsT=qk_t[kb][pbase:pbase + 64, kplane,
                                              kl * P:(kl + 1) * P],
                                rhs=qk_t[qc][pbase:pbase + 64, qplane, c0:QC],
                                start=True,
                                stop=True,
                            )
                        w_t = wtpool.tile([P, 2, QC], BF16, name="w_t", tag="W")
                        nc.scalar.activation(w_t[:, :, c0:], ps_s[:, :, c0:], EXP)
                        if j >= 0:
                            for d in (0, 1):
                                nc.vector.tensor_tensor(
                                    out=w_t[:, d, c0:c0 + P],
                                    in0=w_t[:, d, c0:c0 + P],
                                    in1=mask_sb[:],
                                    op=MUL,
                                )
                        for d in (0, 1):
                            h = 2 * hp + d
                            nc.tensor.matmul(
                                po[0:65, d, c0:QC],
                                lhsT=v_t[kb][:, kl, 65 * h:65 * h + 65],
                                rhs=w_t[:, d, c0:QC],
                                start=(kt == 0),
                                stop=(kt == nkt - 1),
                            )
                        step += 1
                        fill()
                    # denominator inversion on ACT: rec = exp(-ln(den)),
                    # reading the two denominator rows straight from PSUM
                    t2 = small.tile([1, 2, QC], F32, name="t2", tag="T2")
                    nc.scalar.activation(t2[:], po[64:65, :, :], LN)
                    rec2 = small.tile([1, 2, QC], F32, name="rec2", tag="RC")
                    nc.scalar.activation(rec2[:], t2[:], EXP, scale=-1.0)
                    for d in (0, 1):
                        h = 2 * hp + d
                        rbc = small.tile([64, QC], F32, name="rbc", tag="RB")
                        nc.gpsimd.partition_broadcast(rbc[:], rec2[:, d])
                        nc.vector.tensor_tensor(
                            out=o_t[64 * (h % 2):64 * (h % 2) + 64, h // 2, :],
                            in0=po[0:64, d, :],
                            in1=rbc[:],
                            op=MUL,
                        )
                    step += 1
                    fill()
                while done < len(fillers):
                    fillers[done]()
                    done += 1
                return o_t

            for f in phase1_fillers(0):
                f()
            prev = []  # proj fillers from the previous chunk
            for i in range(NQC):
                fillers = prev + (phase1_fillers(i + 1) if i + 1 < NQC else [])
                o_t = attn_block(i, fillers)
                prev = proj_unit(i, o_t)
            for f in prev:
                f()

    print(f"[build] traced+scheduled {_time.time()-_t0:.1f}s", flush=True)
    nc.compile()
    print(f"[build] compiled {_time.time()-_t0:.1f}s", flush=True)
    return nc


def _get_nc():
    if "nc" not in _CACHE:
        _CACHE["nc"] = _build()
    return _CACHE["nc"]


def _make_in_maps(x, W_attn, W_proj):
    import ml_dtypes

    BF = ml_dtypes.bfloat16
    Wp = W_proj.reshape(NH, C, HD)  # [head, dout, d]
    A = Wp.reshape(8, 2, C, HD)     # [pair, hl, dout, d]
    wp_all = np.ascontiguousarray(A.transpose(1, 3, 0, 2).reshape(P, 8, C))
    mask_host = np.triu(np.ones([P, P], dtype=np.float32)).astype(BF)

    in_maps = []
    for core in range(NCORE):
        b, g = core // 4, core % 4
        xt = np.ascontiguousarray(x[b].T).astype(np.float16)  # [C, T]
        Wq = W_attn[256 * g: 256 * (g + 1)]
        Wk = W_attn[C + 256 * g: C + 256 * (g + 1)]
        Wv = W_attn[2 * C + 256 * g: 2 * C + 256 * (g + 1)]
        wqk = np.ascontiguousarray(
            np.concatenate([Wq, Wk], 0).T).astype(np.float16)
        wv = np.ascontiguousarray(Wv.T).astype(np.float16)  # [C, 256]
        wp = np.ascontiguousarray(wp_all[:, 2 * g: 2 * g + 2, :]).astype(BF)
        in_maps.append(
            {"xt": xt, "wqk": wqk, "wv": wv, "wp": wp, "mask": mask_host}
        )
    return in_maps


def kernel(x, W_attn, W_proj):
    from concourse.bass_utils import run_bass_kernel_spmd

    x = np.asarray(x, dtype=np.float32)
    W_attn = np.asarray(W_attn, dtype=np.float32)
    W_proj = np.asarray(W_proj, dtype=np.float32)

    in_maps = _make_in_maps(x, W_attn, W_proj)
    nc = _get_nc()
    res = run_bass_kernel_spmd(nc, in_maps, core_ids=list(range(NCORE)))
    _CACHE["last_result"] = res

    Y = np.empty((B, T, C), dtype=np.float32)
    for b in range(B):
        acc = res.results[4 * b]["out"].astype(np.float32).copy()
        for g in range(1, 4):
            acc += res.results[4 * b + g]["out"]
        Y[b] = acc.T
    return Y


# revision 11
# speedup vs baseline: 1.3833x; 1.0950x over previous
"""Trainium2 8-core Bass kernel for nn_AttentionHPROJ (B=2,T=2048,C=1024,h=16,hd=64).

Sharding: core = 4*b + g owns batch b and heads [4g, 4g+4).
Matmul operands are 16-bit (1 cyc/row on the PE vs 2 cyc/row for f32r):
fp16 for x/Wqk/Wv and Q/K (logit precision), bf16 where exp outputs flow
(w_t, V, O, Wp — exp can reach ~e^18, overflowing fp16). PSUM accumulation
stays fp32. Layout is transposed (dout on partitions, tokens moving):
  - Q^T,K^T = wqk^T @ x^T ; V natural = x @ Wv^T (lhsT = x^T tiles)
  - S^T = K_h Q_h^T per (head, key-tile) into a single-bank PSUM tile;
    exp on ACT (PSUM->SBUF bf16); causal via triangular mask multiply on
    diagonal subtiles; softmax denominator via ones-column appended to V
    (row 64 of the O^T PSUM accumulator).
  - All 4 heads accumulate O^T concurrently (4 PSUM banks), so the chunk
    ends with ONE packed [4,512] reciprocal (DVE cost scales with free-dim
    length only) + per-head gpsimd broadcast + normalize multiply.
  - Each core emits partial Y^T [1024, 2048] (its 4 heads); host sums the
    quad per batch and transposes.
Phase-1 (QKV) work for chunk i+1 and the c_proj of chunk i-1 are spread
through attention block i as PE filler at single-matmul granularity so the
in-order tensor queue never stalls on the normalize chain and the HAM
clock gate stays at 8/8 (2.4 GHz).
"""
import sys

sys.path.insert(0, "/opt/trn_rl_repo")

import numpy as np

B, T, C = 2, 2048, 1024
NH, HD = 16, 64
P = 128
NCORE = 8
QC = 512          # q-chunk width
NQC = T // QC     # 4
KSUB = C // P     # 8

_CACHE = {}


def _build():
    import functools

    import concourse.bass as bass
    import concourse.mybir as mybir
    import concourse.tile as tile
    from concourse import bacc

    F32 = mybir.dt.float32
    BF16 = mybir.dt.bfloat16
    FP16 = mybir.dt.float16
    EXP = mybir.ActivationFunctionType.Exp
    LN = mybir.ActivationFunctionType.Ln
    MUL = mybir.AluOpType.mult

    # Force a single activation-table set: the kernel uses only Exp and Ln,
    # which coexist in natural_log_exp_and_others. Left to itself the
    # table-load pass alternates exp_and_others <-> natural_log (17 loads,
    # ~22us of ACT time + drain stalls). Emptying every other set (indices
    # preserved -- walrus maps act_func_set_id by position) makes the
    # combined set the only eligible choice, so the load hoists to one.
    import concourse.bacc as _bacc_mod
    if not hasattr(_bacc_mod, "_orig_get_activation_tables"):
        _bacc_mod._orig_get_activation_tables = _bacc_mod.get_activation_tables

        def _patched_tables(arch):
            t = _bacc_mod._orig_get_activation_tables(arch)
            exp = mybir.ActivationFunctionType.Exp
            ln = mybir.ActivationFunctionType.Ln
            good = [n for n, f in t.items() if exp in f and ln in f]
            if good:
                keep = good[0]
                return {n: (f if n == keep else set()) for n, f in t.items()}
            return t

        _bacc_mod.get_activation_tables = _patched_tables

    import time as _time
    _t0 = _time.time()
    print("[build] start", flush=True)
    nc = bacc.Bacc("TRN2", target_bir_lowering=False, debug=False, num_devices=NCORE)

    xt_ext = nc.dram_tensor("xt", [C, T], FP16, kind="ExternalInput").ap()
    wqk_ext = nc.dram_tensor("wqk", [C, 512], FP16, kind="ExternalInput").ap()
    wv_ext = nc.dram_tensor("wv", [C, 256], FP16, kind="ExternalInput").ap()
    wp_ext = nc.dram_tensor("wp", [P, 2, C], BF16, kind="ExternalInput").ap()
    mask_ext = nc.dram_tensor("mask", [P, P], BF16, kind="ExternalInput").ap()
    out_ext = nc.dram_tensor("out", [C, T], F32, kind="ExternalOutput").ap()

    with tile.TileContext(nc) as tc:
        with (
            tc.tile_pool(name="wpool", bufs=1) as wpool,
            tc.tile_pool(name="xpool", bufs=1) as xpool,
            tc.tile_pool(name="qkpool", bufs=1) as qkpool,
            tc.tile_pool(name="vpool", bufs=1) as vpool,
            tc.tile_pool(name="wtile", bufs=6) as wtpool,
            tc.tile_pool(name="opool", bufs=2) as opool,
            tc.tile_pool(name="ypool", bufs=2) as ypool,
            tc.tile_pool(name="small", bufs=8) as small,
            tc.tile_pool(name="psA", bufs=2, space="PSUM") as psA,  # 4 banks: S pair
            tc.tile_pool(name="psO", bufs=1, space="PSUM") as psO,  # 2 banks: O pair
            tc.tile_pool(name="psB", bufs=2, space="PSUM") as psB,  # 2 banks: qkv/proj
        ):
            # ---- weights / constants / inputs ----
            wqk_sb = wpool.tile([P, KSUB, 512], FP16)
            wv_sb = wpool.tile([P, KSUB, 256], FP16)
            xt_t = [xpool.tile([P, KSUB, QC], FP16, name=f"xt_{i}", tag=f"xt_{i}")
                    for i in range(NQC)]
            # qk_t planes: 0,1 = Q heads (01),(23); 2,3 = K heads (01),(23);
            # within a plane, partitions 0-63 = even head, 64-127 = odd head.
            qk_t = [qkpool.tile([P, 4, QC], FP16, name=f"qk_{i}", tag=f"qk_{i}")
                    for i in range(NQC)]
            v_t = [vpool.tile([P, 4, 4 * 65], BF16, name=f"v_{i}", tag=f"v_{i}")
                   for i in range(NQC)]
            mask_sb = wpool.tile([P, P], BF16)
            wp_sb = wpool.tile([P, 2, C], BF16)

            wqk_r = wqk_ext.rearrange("(ko p) m -> p ko m", p=P)
            wv_r = wv_ext.rearrange("(ko p) m -> p ko m", p=P)
            xt_r = xt_ext.rearrange("(ko p) t -> p ko t", p=P)
            out_r = out_ext.rearrange("(d p) t -> p d t", p=P)

            # Queue order = need order: tiny constants, then block-0 feed
            # (wqk interleaved with xt chunk 0), then wv/wp, then the rest.
            nc.sync.dma_start(mask_sb[:], mask_ext[:])
            for i in range(NQC):
                ov = v_t[i].rearrange("p t (h c) -> p t h c", c=65)[:, :, :, 64]
                nc.vector.memset(ov, 1.0)
            for k in range(KSUB):
                nc.sync.dma_start(wqk_sb[:, k], wqk_r[:, k])
                nc.sync.dma_start(xt_t[0][:, k], xt_r[:, k, 0:QC])
            nc.sync.dma_start(wv_sb[:], wv_r[:])
            nc.sync.dma_start(wp_sb[:], wp_ext[:])
            for i in range(1, NQC):
                nc.sync.dma_start(xt_t[i][:], xt_r[:, :, i * QC:(i + 1) * QC])

            # ---- phase-1 units (returned as closure lists for filling) ----
            def qk_unit(tc_i, dt):
                st = {}

                def mm(k):
                    if "ps" not in st:
                        st["ps"] = psB.tile([P, QC], F32,
                                            name=f"ps_qk_{tc_i}_{dt}", tag="B")
                    nc.tensor.matmul(
                        st["ps"][:],
                        lhsT=wqk_sb[:, k, dt * P:(dt + 1) * P],
                        rhs=xt_t[tc_i][:, k],
                        start=(k == 0),
                        stop=(k == KSUB - 1),
                    )

                def cp():
                    nc.vector.tensor_copy(out=qk_t[tc_i][:, dt], in_=st["ps"][:])

                return [functools.partial(mm, k) for k in range(KSUB)] + [cp]

            def v_unit(tc_i, tlp):
                st = {}

                def mm(d, k):
                    if "ps" not in st:
                        st["ps"] = psB.tile([P, 2, 256], F32,
                                            name=f"ps_v_{tc_i}_{tlp}", tag="B")
                    tl = 2 * tlp + d
                    nc.tensor.matmul(
                        st["ps"][:, d],
                        lhsT=xt_t[tc_i][:, k, tl * P:(tl + 1) * P],
                        rhs=wv_sb[:, k],
                        start=(k == 0),
                        stop=(k == KSUB - 1),
                    )

                def cp():
                    vdst = v_t[tc_i].rearrange("p t (h c) -> p t h c", c=65)[
                        :, 2 * tlp:2 * tlp + 2, :, 0:64
                    ]
                    nc.vector.tensor_copy(
                        out=vdst,
                        in_=st["ps"].rearrange("p t (h d) -> p t h d", d=64),
                    )

                return [functools.partial(mm, d, k)
                        for d in (0, 1) for k in range(KSUB)] + [cp]

            def phase1_fillers(tc_i):
                fs = []
                for dt in (0, 2, 1, 3):
                    fs += qk_unit(tc_i, dt)
                for tlp in (0, 1):
                    fs += v_unit(tc_i, tlp)
                return fs

            # ---- c_proj unit for one finished chunk (as filler closures) ----
            def proj_unit(qc, o_t):
                st = {}
                fs = []

                def mm(dt, pr):
                    if dt not in st:
                        st[dt] = psB.tile([P, QC], F32,
                                          name=f"ps_y_{qc}_{dt}", tag="B")
                    nc.tensor.matmul(
                        st[dt][:],
                        lhsT=wp_sb[:, pr, dt * P:(dt + 1) * P],
                        rhs=o_t[:, pr, :],
                        start=(pr == 0),
                        stop=(pr == 1),
                    )

                def cp(dt):
                    nc.vector.tensor_copy(out=st["y"][:, dt], in_=st[dt][:])

                def alloc_y():
                    st["y"] = ypool.tile([P, 8, QC], F32, name=f"y_{qc}", tag="Y")

                fs.append(alloc_y)
                for dt in range(8):
                    fs.append(functools.partial(mm, dt, 0))
                    fs.append(functools.partial(mm, dt, 1))
                    fs.append(functools.partial(cp, dt))

                def dma():
                    nc.sync.dma_start(out_r[:, :, qc * QC:(qc + 1) * QC], st["y"][:])

                fs.append(dma)
                return fs

            # ---- attention ----
            # hp pair loop: S for both heads of a partition-pair land in one
            # [P,2,QC] PSUM mega-tile (2 banks) -> ONE 1024-wide exp; O^T for
            # the pair accumulates in a [P,2,QC] pair-mega; the softmax
            # denominators (row 64 of both planes) are inverted entirely on
            # ACT via exp(-ln(x)) read straight from PSUM.
            def attn_block(qc, fillers):
                o_t = opool.tile([P, 2, QC], BF16, name=f"o_{qc}", tag="OT")
                nkt = 4 * qc + 4
                nsteps = 2 * (nkt + 1)
                done = 0
                step = 0

                def fill():
                    nonlocal done
                    want = step * len(fillers) // nsteps
                    while done < want:
                        fillers[done]()
                        done += 1

                for hp in range(2):
                    po = psO.tile([P, 2, QC], F32, name=f"ps_o_{qc}_{hp}", tag="O")
                    for kt in range(nkt):
                        j = kt - 4 * qc
                        c0 = max(0, j) * P
                        kb, kl = kt // 4, kt % 4
                        ps_s = psA.tile([P, 2, QC], F32,
                                        name=f"ps_s_{qc}_{hp}_{kt}", tag="A")
                        for d in (0, 1):
                            h = 2 * hp + d
                            pbase = 64 * (h % 2)
                            qplane = h // 2
                            kplane = 2 + h // 2
                            nc.tensor.matmul(
                                ps_s[:, d, c0:QC],
                                lhsT=qk_t[kb][pbase:pbase + 64, kplane,
                                              kl * P:(kl + 1) * P],
                                rhs=qk_t[qc][pbase:pbase + 64, qplane, c0:QC],
                                start=True,
                                stop=True,
                            )
                        w_t = wtpool.tile([P, 2, QC], BF16, name="w_t", tag="W")
                        nc.scalar.activation(w_t[:, :, c0:], ps_s[:, :, c0:], EXP)
                        if j >= 0:
                            for d in (0, 1):
                                nc.vector.tensor_tensor(
                                    out=w_t[:, d, c0:c0 + P],
                                    in0=w_t[:, d, c0:c0 + P],
                                    in1=mask_sb[:],
                                    op=MUL,
                                )
                        for d in (0, 1):
                            h = 2 * hp + d
                            nc.tensor.matmul(
                                po[0:65, d, c0:QC],
                                lhsT=v_t[kb][:, kl, 65 * h:65 * h + 65],
                                rhs=w_t[:, d, c0:QC],
                                start=(kt == 0),
                                stop=(kt == nkt - 1),
                            )
                        step += 1
                        fill()
                    # denominator inversion on ACT: rec = exp(-ln(den)),
                    # reading the two denominator rows straight from PSUM
                    t2 = small.tile([1, 2, QC], F32, name="t2", tag="T2")
                    nc.scalar.activation(t2[:], po[64:65, :, :], LN)
                    rec2 = small.tile([1, 2, QC], F32, name="rec2", tag="RC")
                    nc.scalar.activation(rec2[:], t2[:], EXP, scale=-1.0)
                    for d in (0, 1):
                        h = 2 * hp + d
                        rbc = small.tile([64, QC], F32, name="rbc", tag="RB")
                        nc.gpsimd.partition_broadcast(rbc[:], rec2[:, d])
                        nc.vector.tensor_tensor(
                            out=o_t[64 * (h % 2):64 * (h % 2) + 64, h // 2, :],
                            in0=po[0:64, d, :],
                            in1=rbc[:],
                            op=MUL,
                        )
                    step += 1
                    fill()
                while done < len(fillers):
                    fillers[done]()
                    done += 1
                return o_t

            for f in phase1_fillers(0):
                f()
            prev = []  # proj fillers from the previous chunk
            for i in range(NQC):
                fillers = prev + (phase1_fillers(i + 1) if i + 1 < NQC else [])
                o_t = attn_block(i, fillers)
                prev = proj_unit(i, o_t)
            for f in prev:
                f()

    print(f"[build] traced+scheduled {_time.time()-_t0:.1f}s", flush=True)
    nc.compile()
    print(f"[build] compiled {_time.time()-_t0:.1f}s", flush=True)
    return nc


def _get_nc():
    if "nc" not in _CACHE:
        _CACHE["nc"] = _build()
    return _CACHE["nc"]


def _make_in_maps(x, W_attn, W_proj):
    import ml_dtypes

    BF = ml_dtypes.bfloat16
    Wp = W_proj.reshape(NH, C, HD)  # [head, dout, d]
    A = Wp.reshape(8, 2, C, HD)     # [pair, hl, dout, d]
    wp_all = np.ascontiguousarray(A.transpose(1, 3, 0, 2).reshape(P, 8, C))
    mask_host = np.triu(np.ones([P, P], dtype=np.float32)).astype(BF)

    in_maps = []
    for core in range(NCORE):
        b, g = core // 4, core % 4
        xt = np.ascontiguousarray(x[b].T).astype(np.float16)  # [C, T]
        Wq = W_attn[256 * g: 256 * (g + 1)]
        Wk = W_attn[C + 256 * g: C + 256 * (g + 1)]
        Wv = W_attn[2 * C + 256 * g: 2 * C + 256 * (g + 1)]
        wqk = np.ascontiguousarray(
            np.concatenate([Wq, Wk], 0).T).astype(np.float16)
        wv = np.ascontiguousarray(Wv.T).astype(np.float16)  # [C, 256]
        wp = np.ascontiguousarray(wp_all[:, 2 * g: 2 * g + 2, :]).astype(BF)
        in_maps.append(
            {"xt": xt, "wqk": wqk, "wv": wv, "wp": wp, "mask": mask_host}
        )
    return in_maps


def kernel(x, W_attn, W_proj):
    from concourse.bass_utils import run_bass_kernel_spmd

    x = np.asarray(x, dtype=np.float32)
    W_attn = np.asarray(W_attn, dtype=np.float32)
    W_proj = np.asarray(W_proj, dtype=np.float32)

    in_maps = _make_in_maps(x, W_attn, W_proj)
    nc = _get_nc()
    res = run_bass_kernel_spmd(nc, in_maps, core_ids=list(range(NCORE)))
    _CACHE["last_result"] = res

    Y = np.empty((B, T, C), dtype=np.float32)
    for b in range(B):
        acc = res.results[4 * b]["out"].astype(np.float32).copy()
        for g in range(1, 4):
            acc += res.results[4 * b + g]["out"]
        Y[b] = acc.T
    return Y


# revision 13
# speedup vs baseline: 1.3963x; 1.0094x over previous
"""Trainium2 8-core Bass kernel for nn_AttentionHPROJ (B=2,T=2048,C=1024,h=16,hd=64).

Sharding: core = 4*b + g owns batch b and heads [4g, 4g+4).
Matmul operands are 16-bit (1 cyc/row on the PE vs 2 cyc/row for f32r):
fp16 for x/Wqk/Wv and Q/K (logit precision), bf16 where exp outputs flow
(w_t, V, O, Wp — exp can reach ~e^18, overflowing fp16). PSUM accumulation
stays fp32. Layout is transposed (dout on partitions, tokens moving):
  - Q^T,K^T = wqk^T @ x^T ; V natural = x @ Wv^T (lhsT = x^T tiles)
  - S^T = K_h Q_h^T per (head, key-tile) into a single-bank PSUM tile;
    exp on ACT (PSUM->SBUF bf16); causal via triangular mask multiply on
    diagonal subtiles; softmax denominator via ones-column appended to V
    (row 64 of the O^T PSUM accumulator).
  - All 4 heads accumulate O^T concurrently (4 PSUM banks), so the chunk
    ends with ONE packed [4,512] reciprocal (DVE cost scales with free-dim
    length only) + per-head gpsimd broadcast + normalize multiply.
  - Each core emits partial Y^T [1024, 2048] (its 4 heads); host sums the
    quad per batch and transposes.
Phase-1 (QKV) work for chunk i+1 and the c_proj of chunk i-1 are spread
through attention block i as PE filler at single-matmul granularity so the
in-order tensor queue never stalls on the normalize chain and the HAM
clock gate stays at 8/8 (2.4 GHz).
"""
import sys

sys.path.insert(0, "/opt/trn_rl_repo")

import numpy as np

B, T, C = 2, 2048, 1024
NH, HD = 16, 64
P = 128
NCORE = 8
QC = 512          # q-chunk width
NQC = T // QC     # 4
KSUB = C // P     # 8

_CACHE = {}


def _build():
    import functools

    import concourse.bass as bass
    import concourse.mybir as mybir
    import concourse.tile as tile
    from concourse import bacc

    F32 = mybir.dt.float32
    BF16 = mybir.dt.bfloat16
    FP16 = mybir.dt.float16
    EXP = mybir.ActivationFunctionType.Exp
    LN = mybir.ActivationFunctionType.Ln
    MUL = mybir.AluOpType.mult

    # Force a single activation-table set: the kernel uses only Exp and Ln,
    # which coexist in natural_log_exp_and_others. Left to itself the
    # table-load pass alternates exp_and_others <-> natural_log (17 loads,
    # ~22us of ACT time + drain stalls). Emptying every other set (indices
    # preserved -- walrus maps act_func_set_id by position) makes the
    # combined set the only eligible choice, so the load hoists to one.
    import concourse.bacc as _bacc_mod
    if not hasattr(_bacc_mod, "_orig_get_activation_tables"):
        _bacc_mod._orig_get_activation_tables = _bacc_mod.get_activation_tables

        def _patched_tables(arch):
            t = _bacc_mod._orig_get_activation_tables(arch)
            exp = mybir.ActivationFunctionType.Exp
            ln = mybir.ActivationFunctionType.Ln
            good = [n for n, f in t.items() if exp in f and ln in f]
            if good:
                keep = good[0]
                return {n: (f if n == keep else set()) for n, f in t.items()}
            return t

        _bacc_mod.get_activation_tables = _patched_tables

    import time as _time
    _t0 = _time.time()
    print("[build] start", flush=True)
    nc = bacc.Bacc("TRN2", target_bir_lowering=False, debug=False, num_devices=NCORE)

    xt_ext = nc.dram_tensor("xt", [C, T], FP16, kind="ExternalInput").ap()
    wqk_ext = nc.dram_tensor("wqk", [C, 512], FP16, kind="ExternalInput").ap()
    wv_ext = nc.dram_tensor("wv", [C, 256], FP16, kind="ExternalInput").ap()
    wp_ext = nc.dram_tensor("wp", [P, 2, C], BF16, kind="ExternalInput").ap()
    mask_ext = nc.dram_tensor("mask", [P, P], BF16, kind="ExternalInput").ap()
    out_ext = nc.dram_tensor("out", [C, T], F32, kind="ExternalOutput").ap()

    with tile.TileContext(nc) as tc:
        with (
            tc.tile_pool(name="wpool", bufs=1) as wpool,
            tc.tile_pool(name="xpool", bufs=1) as xpool,
            tc.tile_pool(name="qkpool", bufs=1) as qkpool,
            tc.tile_pool(name="vpool", bufs=1) as vpool,
            tc.tile_pool(name="wtile", bufs=6) as wtpool,
            tc.tile_pool(name="opool", bufs=2) as opool,
            tc.tile_pool(name="ypool", bufs=2) as ypool,
            tc.tile_pool(name="small", bufs=8) as small,
            tc.tile_pool(name="psA", bufs=2, space="PSUM") as psA,  # 4 banks: S pair
            tc.tile_pool(name="psO", bufs=1, space="PSUM") as psO,  # 2 banks: O pair
            tc.tile_pool(name="psB", bufs=2, space="PSUM") as psB,  # 2 banks: qkv/proj
        ):
            # ---- weights / constants / inputs ----
            wqk_sb = wpool.tile([P, KSUB, 512], FP16)
            wv_sb = wpool.tile([P, KSUB, 256], FP16)
            xt_t = [xpool.tile([P, KSUB, QC], FP16, name=f"xt_{i}", tag=f"xt_{i}")
                    for i in range(NQC)]
            # qk_t planes: 0,1 = Q heads (01),(23); 2,3 = K heads (01),(23);
            # within a plane, partitions 0-63 = even head, 64-127 = odd head.
            qk_t = [qkpool.tile([P, 4, QC], FP16, name=f"qk_{i}", tag=f"qk_{i}")
                    for i in range(NQC)]
            v_t = [vpool.tile([P, 4, 4 * 65], BF16, name=f"v_{i}", tag=f"v_{i}")
                   for i in range(NQC)]
            mask_sb = wpool.tile([P, P], BF16)
            wp_sb = wpool.tile([P, 2, C], BF16)

            wqk_r = wqk_ext.rearrange("(ko p) m -> p ko m", p=P)
            wv_r = wv_ext.rearrange("(ko p) m -> p ko m", p=P)
            xt_r = xt_ext.rearrange("(ko p) t -> p ko t", p=P)
            out_r = out_ext.rearrange("(d p) t -> p d t", p=P)

            # Queue order = need order: tiny constants, then block-0 feed
            # (wqk interleaved with xt chunk 0), then wv/wp, then the rest.
            for k in range(KSUB):
                nc.sync.dma_start(wqk_sb[:, k], wqk_r[:, k])
                nc.sync.dma_start(xt_t[0][:, k], xt_r[:, k, 0:QC])
            nc.sync.dma_start(mask_sb[:], mask_ext[:])
            for i in range(NQC):
                ov = v_t[i].rearrange("p t (h c) -> p t h c", c=65)[:, :, :, 64]
                nc.vector.memset(ov, 1.0)
            nc.sync.dma_start(wv_sb[:], wv_r[:])
            nc.sync.dma_start(wp_sb[:], wp_ext[:])
            for i in range(1, NQC):
                nc.sync.dma_start(xt_t[i][:], xt_r[:, :, i * QC:(i + 1) * QC])

            # ---- phase-1 units (returned as closure lists for filling) ----
            def qk_unit(tc_i, dt):
                st = {}

                def mm(k):
                    if "ps" not in st:
                        st["ps"] = psB.tile([P, QC], F32,
                                            name=f"ps_qk_{tc_i}_{dt}", tag="B")
                    nc.tensor.matmul(
                        st["ps"][:],
                        lhsT=wqk_sb[:, k, dt * P:(dt + 1) * P],
                        rhs=xt_t[tc_i][:, k],
                        start=(k == 0),
                        stop=(k == KSUB - 1),
                    )

                def cp():
                    nc.vector.tensor_copy(out=qk_t[tc_i][:, dt], in_=st["ps"][:])

                return [functools.partial(mm, k) for k in range(KSUB)] + [cp]

            def v_unit(tc_i, tlp):
                st = {}

                def mm(d, k):
                    if "ps" not in st:
                        st["ps"] = psB.tile([P, 2, 256], F32,
                                            name=f"ps_v_{tc_i}_{tlp}", tag="B")
                    tl = 2 * tlp + d
                    nc.tensor.matmul(
                        st["ps"][:, d],
                        lhsT=xt_t[tc_i][:, k, tl * P:(tl + 1) * P],
                        rhs=wv_sb[:, k],
                        start=(k == 0),
                        stop=(k == KSUB - 1),
                    )

                def cp():
                    vdst = v_t[tc_i].rearrange("p t (h c) -> p t h c", c=65)[
                        :, 2 * tlp:2 * tlp + 2, :, 0:64
                    ]
                    nc.vector.tensor_copy(
                        out=vdst,
                        in_=st["ps"].rearrange("p t (h d) -> p t h d", d=64),
                    )

                return [functools.partial(mm, d, k)
                        for d in (0, 1) for k in range(KSUB)] + [cp]

            def phase1_fillers(tc_i):
                fs = []
                for dt in (0, 2, 1, 3):
                    fs += qk_unit(tc_i, dt)
                for tlp in (0, 1):
                    fs += v_unit(tc_i, tlp)
                return fs

            # ---- c_proj unit for one finished chunk (as filler closures) ----
            def proj_unit(qc, o_t):
                st = {}
                fs = []

                def mm(dt, pr):
                    if dt not in st:
                        st[dt] = psB.tile([P, QC], F32,
                                          name=f"ps_y_{qc}_{dt}", tag="B")
                    nc.tensor.matmul(
                        st[dt][:],
                        lhsT=wp_sb[:, pr, dt * P:(dt + 1) * P],
                        rhs=o_t[:, pr, :],
                        start=(pr == 0),
                        stop=(pr == 1),
                    )

                def cp(dt):
                    nc.vector.tensor_copy(out=st["y"][:, dt], in_=st[dt][:])
                    nc.sync.dma_start(
                        out_r[:, dt, qc * QC:(qc + 1) * QC], st["y"][:, dt]
                    )

                def alloc_y():
                    st["y"] = ypool.tile([P, 8, QC], F32, name=f"y_{qc}", tag="Y")

                fs.append(alloc_y)
                for dt in range(8):
                    fs.append(functools.partial(mm, dt, 0))
                    fs.append(functools.partial(mm, dt, 1))
                    fs.append(functools.partial(cp, dt))

                return fs

            # ---- attention ----
            # hp pair loop: S for both heads of a partition-pair land in one
            # [P,2,QC] PSUM mega-tile (2 banks) -> ONE 1024-wide exp; O^T for
            # the pair accumulates in a [P,2,QC] pair-mega; the softmax
            # denominators (row 64 of both planes) are inverted entirely on
            # ACT via exp(-ln(x)) read straight from PSUM.
            def attn_block(qc, fillers):
                o_t = opool.tile([P, 2, QC], BF16, name=f"o_{qc}", tag="OT")
                nkt = 4 * qc + 4
                nsteps = 2 * (nkt + 1)
                done = 0
                step = 0

                def fill():
                    nonlocal done
                    want = step * len(fillers) // nsteps
                    while done < want:
                        fillers[done]()
                        done += 1

                for hp in range(2):
                    po = psO.tile([P, 2, QC], F32, name=f"ps_o_{qc}_{hp}", tag="O")
                    for kt in range(nkt):
                        j = kt - 4 * qc
                        c0 = max(0, j) * P
                        kb, kl = kt // 4, kt % 4
                        ps_s = psA.tile([P, 2, QC], F32,
                                        name=f"ps_s_{qc}_{hp}_{kt}", tag="A")
                        for d in (0, 1):
                            h = 2 * hp + d
                            pbase = 64 * (h % 2)
                            qplane = h // 2
                            kplane = 2 + h // 2
                            nc.tensor.matmul(
                                ps_s[:, d, c0:QC],
                                lhsT=qk_t[kb][pbase:pbase + 64, kplane,
                                              kl * P:(kl + 1) * P],
                                rhs=qk_t[qc][pbase:pbase + 64, qplane, c0:QC],
                                start=True,
                                stop=True,
                            )
                        w_t = wtpool.tile([P, 2, QC], BF16, name="w_t", tag="W")
                        nc.scalar.activation(w_t[:, :, c0:], ps_s[:, :, c0:], EXP)
                        if j >= 0:
                            for d in (0, 1):
                                nc.vector.tensor_tensor(
                                    out=w_t[:, d, c0:c0 + P],
                                    in0=w_t[:, d, c0:c0 + P],
                                    in1=mask_sb[:],
                                    op=MUL,
                                )
                        for d in (0, 1):
                            h = 2 * hp + d
                            nc.tensor.matmul(
                                po[0:65, d, c0:QC],
                                lhsT=v_t[kb][:, kl, 65 * h:65 * h + 65],
                                rhs=w_t[:, d, c0:QC],
                                start=(kt == 0),
                                stop=(kt == nkt - 1),
                            )
                        step += 1
                        fill()
                    # denominator inversion on ACT: rec = exp(-ln(den)),
                    # reading the two denominator rows straight from PSUM
                    t2 = small.tile([1, 2, QC], F32, name="t2", tag="T2")
                    nc.scalar.activation(t2[:], po[64:65, :, :], LN)
                    rec2 = small.tile([1, 2, QC], F32, name="rec2", tag="RC")
                    nc.scalar.activation(rec2[:], t2[:], EXP, scale=-1.0)
                    for d in (0, 1):
                        h = 2 * hp + d
                        rbc = small.tile([64, QC], F32, name="rbc", tag="RB")
                        nc.gpsimd.partition_broadcast(rbc[:], rec2[:, d])
                        nc.vector.tensor_tensor(
                            out=o_t[64 * (h % 2):64 * (h % 2) + 64, h // 2, :],
                            in0=po[0:64, d, :],
                            in1=rbc[:],
                            op=MUL,
                        )
                    step += 1
                    fill()
                while done < len(fillers):
                    fillers[done]()
                    done += 1
                return o_t

            for f in phase1_fillers(0):
                f()
            prev = []  # proj fillers from the previous chunk
            for i in range(NQC):
                fillers = prev + (phase1_fillers(i + 1) if i + 1 < NQC else [])
                o_t = attn_block(i, fillers)
                prev = proj_unit(i, o_t)
            for f in prev:
                f()

    print(f"[build] traced+scheduled {_time.time()-_t0:.1f}s", flush=True)
    nc.compile()
    print(f"[build] compiled {_time.time()-_t0:.1f}s", flush=True)
    return nc


def _get_nc():
    if "nc" not in _CACHE:
        _CACHE["nc"] = _build()
    return _CACHE["nc"]


def _make_in_maps(x, W_attn, W_proj):
    import ml_dtypes

    BF = ml_dtypes.bfloat16
    Wp = W_proj.reshape(NH, C, HD)  # [head, dout, d]
    A = Wp.reshape(8, 2, C, HD)     # [pair, hl, dout, d]
    wp_all = np.ascontiguousarray(A.transpose(1, 3, 0, 2).reshape(P, 8, C))
    mask_host = np.triu(np.ones([P, P], dtype=np.float32)).astype(BF)

    in_maps = []
    for core in range(NCORE):
        b, g = core // 4, core % 4
        xt = np.ascontiguousarray(x[b].T).astype(np.float16)  # [C, T]
        Wq = W_attn[256 * g: 256 * (g + 1)]
        Wk = W_attn[C + 256 * g: C + 256 * (g + 1)]
        Wv = W_attn[2 * C + 256 * g: 2 * C + 256 * (g + 1)]
        wqk = np.ascontiguousarray(
            np.concatenate([Wq, Wk], 0).T).astype(np.float16)
        wv = np.ascontiguousarray(Wv.T).astype(np.float16)  # [C, 256]
        wp = np.ascontiguousarray(wp_all[:, 2 * g: 2 * g + 2, :]).astype(BF)
        in_maps.append(
            {"xt": xt, "wqk": wqk, "wv": wv, "wp": wp, "mask": mask_host}
        )
    return in_maps


def kernel(x, W_attn, W_proj):
    from concourse.bass_utils import run_bass_kernel_spmd

    x = np.asarray(x, dtype=np.float32)
    W_attn = np.asarray(W_attn, dtype=np.float32)
    W_proj = np.asarray(W_proj, dtype=np.float32)

    in_maps = _make_in_maps(x, W_attn, W_proj)
    nc = _get_nc()
    res = run_bass_kernel_spmd(nc, in_maps, core_ids=list(range(NCORE)))
    _CACHE["last_result"] = res

    Y = np.empty((B, T, C), dtype=np.float32)
    for b in range(B):
        acc = res.results[4 * b]["out"].astype(np.float32).copy()
        for g in range(1, 4):
            acc += res.results[4 * b + g]["out"]
        Y[b] = acc.T
    return Y
